# revision 13
# baseline (speedup 1.0000x reference)
"""Bass/Tile Trainium2 kernel for CrossPositionalAttention.

Reference math (per batch element b):
    M = F @ W_M; N = F @ W_N; V = F @ W_V          # [T, C] each, T=2048, C=64
    S = softmax(M @ N^T, axis=-1)                  # [T, T]
    out = S @ V + F

Sharding: data-parallel over batch. B=8 == n_cores=8, so core i computes
batch element i end-to-end (no collectives); kernel() shards/gathers on host.

Per-core dataflow (P=128 partitions), v3:
  F_sb [128,16,64] f32 natural tiles (DMA), F^T via 16 PE transposes
  projections: single-pass f32r matmuls (~12-bit operand mantissa keeps
    score error tiny; f32r streams 1 PE cycle/col at 512 free-dim when the
    clock is warm -- hw-verified).  M^T/N^T [128,2048] f32r duplicated on
    both partition halves via [W|W] stationaries.
  scores^T tile [k=128, q=1024...]: per (qc,kp): two k-blocks row-packed
    concurrently (tile_position (0,0)/(64,0)) -> sc [128,1024] fp32 PSUM.
  exp split across both post-PE engines to beat the single-engine floor:
    even tiles: ACT  expS = exp(sc - 40) -> bf16   (~1113ns/tile)
    odd tiles:  DVE Schraudolph one-op exp: u16 = rne_sat_u16(sc*A + B)
      bitcast bf16 ~= exp(sc-40); negatives saturate to +0.0 (hw-checked).
  attention+V folded: instead of projecting V up front, accumulate
    U' = [F|1|0]^T @ expS  [66,512] fp32 (stationary = bf16 F block with a
    ones column -> row 64 = softmax denominator), then one small matmul
    out2 = Wv'^T @ U' with Wv' = [[Wv,0],[0,1],[0,0]] applies W_V after
    attention (linearity).  This kills the V projection, its PSUM->SBUF
    copies, and the main loop's dependency on it.
  epilogue per q-chunk: DVE copy U'->bf16, PE out2 matmul, DVE copy ->
    bf16, 4 PE transposes into one PSUM bank, DVE strided reciprocal of
    the 4 denominator cols, ACT per-block scale (Copy activation with
    per-partition scale AP), DVE batched residual add, DMA out.
"""

import math

import numpy as np

import concourse.bacc as bacc
import concourse.bass as bass
import concourse.tile as tile
from concourse import mybir
from concourse.bass_utils import run_bass_kernel_spmd
from concourse.masks import make_identity

B, T, C = 8, 2048, 64
P = 128
NBLK = T // P          # 16 k-blocks (and q-blocks) of 128
QCHUNK = 512           # moving-operand free dim per matmul
NQC = T // QCHUNK      # 4 q-chunks
F32 = mybir.dt.float32
F32R = mybir.dt.float32r
BF16 = mybir.dt.bfloat16
U16 = mybir.dt.uint16
EXP_BIAS = -40.0       # constant softmax shift (cancels in the normalization)
VPAD = 66              # U' rows: 64 channels + denominator + zero pad

# Schraudolph exp constants for the DVE path: u16(x*A + B) bitcast bf16
# approximates exp(x + EXP_BIAS).
_C_ADJ = 150000.0
SCH_A = (2.0**23) / math.log(2.0) / 65536.0
SCH_B = (127.0 * 2.0**23 - _C_ADJ) / 65536.0 + EXP_BIAS * SCH_A


def build_nc() -> bass.Bass:
    nc = bacc.Bacc()
    F_h = nc.declare_dram_parameter("F", [T, C], F32, isOutput=False)
    Wm_h = nc.declare_dram_parameter("W_M", [C, C], F32, isOutput=False)
    Wn_h = nc.declare_dram_parameter("W_N", [C, C], F32, isOutput=False)
    Wv_h = nc.declare_dram_parameter("W_V", [C, C], F32, isOutput=False)
    out_h = nc.declare_dram_parameter("out", [T, C], F32, isOutput=True)

    # [T, C] viewed as [128, 16, C]: partition p, block n -> row n*128 + p
    F_view = F_h[:, :].rearrange("(n p) c -> p n c", p=P)
    out_view = out_h[:, :].rearrange("(n p) c -> p n c", p=P)

    with tile.TileContext(nc) as tc:
        with (
            tc.tile_pool(name="const", bufs=1) as const_pool,
            tc.tile_pool(name="persist", bufs=1) as persist,
        ):
            ident = const_pool.tile([P, P], F32, tag="ident")
            make_identity(nc, ident)

            exp_bias = const_pool.tile([P, 1], F32, tag="expbias")
            nc.vector.memset(exp_bias, EXP_BIAS)
            # force the exp table set to load before any other ACT work
            scratch = const_pool.tile([P, 1], F32, tag="scratch")
            nc.scalar.activation(
                scratch, exp_bias, mybir.ActivationFunctionType.Exp
            )

            # F natural tiles: gate everything, so DMA them first, split
            # across the two HWDGE issue queues (SP + ACT)
            F_sb = persist.tile([P, NBLK, C], F32, tag="fsb")
            for i in range(8):
                q = nc.sync if i % 2 == 0 else nc.scalar
                q.dma_start(
                    out=F_sb[:, 2 * i : 2 * i + 2, :],
                    in_=F_view[:, 2 * i : 2 * i + 2, :],
                )

            # dummy back-to-back matmul burst: forces the PE HAM clock gate
            # to 8/8 (2.4 GHz) while the DMAs land, so the real work never
            # runs at the cold 1.2 GHz clock
            warm_sb = const_pool.tile([P, QCHUNK], BF16, tag="warmsb")
            nc.vector.memset(warm_sb, 0.0)

            # f32r matmul operands must be produced rounded-to-f32r by an
            # engine op, so DMA lands in fp32 staging and DVE copy-rounds.
            Wst = const_pool.tile([C, 3 * C], F32, tag="wst")
            nc.sync.dma_start(out=Wst[:, 0:C], in_=Wm_h[:, :])
            nc.sync.dma_start(out=Wst[:, C : 2 * C], in_=Wn_h[:, :])
            nc.sync.dma_start(out=Wst[:, 2 * C : 3 * C], in_=Wv_h[:, :])
            Wall = const_pool.tile([C, 2 * P], F32R, tag="wall")
            nc.vector.tensor_copy(Wall[:, 0:C], Wst[:, 0:C])
            nc.vector.tensor_copy(Wall[:, C : 2 * C], Wst[:, 0:C])
            nc.vector.tensor_copy(Wall[:, 2 * C : 3 * C], Wst[:, C : 2 * C])
            nc.vector.tensor_copy(Wall[:, 3 * C : 4 * C], Wst[:, C : 2 * C])
            Wm2 = Wall[:, 0:P]
            Wn2 = Wall[:, P : 2 * P]
            # Wv' = [[Wv, 0], [0, 1], [0, 0]]  (applies W_V after attention
            # and carries the denominator row through)
            Wv_b = const_pool.tile([VPAD, VPAD], BF16, tag="wvb")
            nc.vector.memset(Wv_b, 0.0)
            nc.vector.tensor_copy(Wv_b[0:C, 0:C], Wst[:, 2 * C : 3 * C])
            nc.vector.memset(Wv_b[C : C + 1, C : C + 1], 1.0)

            # bf16 [F | 1 | 0] stationary blocks for the U' accumulation
            Fb = persist.tile([P, NBLK, VPAD], BF16, tag="fb")
            nc.vector.memset(Fb[:, :, C:VPAD], 0.0)
            nc.vector.memset(Fb[:, :, C : C + 1], 1.0)
            for i in range(4):
                nc.vector.tensor_copy(
                    Fb[:, 4 * i : 4 * i + 4, 0:C],
                    F_sb[:, 4 * i : 4 * i + 4, :],
                )

            F_T = persist.tile([C, T], F32R, tag="ft")
            MT = persist.tile([P, T], F32R, tag="mt")
            NT = persist.tile([P, T], F32R, tag="nt")

            with tc.tile_pool(name="pre_ps", bufs=2, space="PSUM") as pre_ps:
                warm_ps = pre_ps.tile([P, QCHUNK], F32, tag="warm")
                for _ in range(12):
                    nc.tensor.matmul(
                        warm_ps,
                        lhsT=warm_sb[:, 0:P],
                        rhs=warm_sb,
                        start=True,
                        stop=True,
                    )

                # Preamble ordered chunk-first so the critical chain to the
                # first scores matmul (transpose c0 -> F_T copy c0 -> proj
                # M/N c0 -> MT/NT copies c0) finishes as early as possible.
                # Copies alternate ACT/DVE to split the load.
                for c in range(NQC):
                    sl = slice(c * QCHUNK, (c + 1) * QCHUNK)
                    tp = pre_ps.tile([C, QCHUNK], F32, tag="tp")
                    for j in range(4):
                        nc.tensor.transpose(
                            tp[:, j * P : (j + 1) * P],
                            F_sb[:, 4 * c + j, :],
                            ident,
                        )
                    if c % 2 == 0:
                        nc.scalar.copy(F_T[:, sl], tp)
                    else:
                        nc.vector.tensor_copy(F_T[:, sl], tp)
                    for pi, (W2, dst) in enumerate(((Wm2, MT), (Wn2, NT))):
                        pp = pre_ps.tile([P, QCHUNK], F32, tag="proj")
                        nc.tensor.matmul(
                            pp,
                            lhsT=W2,
                            rhs=F_T[:, sl],
                            start=True,
                            stop=True,
                        )
                        if pi == 0:
                            nc.vector.tensor_copy(dst[:, sl], pp)
                        else:
                            nc.scalar.copy(dst[:, sl], pp)

                # keep the PE hot while the last MT/NT copies drain on the
                # ACT/DVE queues (a >3.4us PE idle would re-throttle HAM)
                for _ in range(8):
                    nc.tensor.matmul(
                        warm_ps,
                        lhsT=warm_sb[:, 0:P],
                        rhs=warm_sb,
                        start=True,
                        stop=True,
                    )

            with (
                tc.tile_pool(name="sc_ps", bufs=3, space="PSUM") as sc_pool,
                tc.tile_pool(name="u_ps", bufs=1, space="PSUM") as u_pool,
                tc.tile_pool(name="o2_ps", bufs=1, space="PSUM") as o2_pool,
                tc.tile_pool(name="work", bufs=4) as work,
                tc.tile_pool(name="ep", bufs=2) as ep,
            ):
                # software-pipelined by DEPTH tiles: the U' accumulation for
                # tile i-DEPTH issues after the scores matmuls of tile i, so
                # the PE never waits on the just-issued exp of tile i.
                DEPTH = 3
                NT_TILES = NQC * (NBLK // 2)
                exp_tiles = [None] * NT_TILES
                u_tiles = {}

                def emit_scores(i):
                    qc, kp = divmod(i, NBLK // 2)
                    qsl = slice(qc * QCHUNK, (qc + 1) * QCHUNK)
                    sc = sc_pool.tile([P, 2 * QCHUNK], F32, tag="sc")
                    for half in range(2):
                        kblk = 2 * kp + half
                        rows = slice(half * C, half * C + C)
                        ksl = slice(kblk * P, (kblk + 1) * P)
                        bank = slice(half * QCHUNK, (half + 1) * QCHUNK)
                        nc.tensor.matmul(
                            sc[:, bank],
                            lhsT=NT[rows, ksl],
                            rhs=MT[rows, qsl],
                            start=True,
                            stop=True,
                            tile_position=(half * C, 0),
                        )
                    expS = work.tile([P, 2 * QCHUNK], BF16, tag="exps")
                    if i % 2 == 0 or i % 16 == 9:
                        nc.scalar.activation(
                            expS,
                            sc,
                            mybir.ActivationFunctionType.Exp,
                            bias=exp_bias,
                            scale=1.0,
                        )
                    else:
                        nc.vector.tensor_scalar(
                            out=expS[:, :].bitcast(U16),
                            in0=sc,
                            scalar1=SCH_A,
                            scalar2=SCH_B,
                            op0=mybir.AluOpType.mult,
                            op1=mybir.AluOpType.add,
                        )
                    exp_tiles[i] = expS

                def emit_u(i):
                    qc, kp = divmod(i, NBLK // 2)
                    if kp == 0:
                        u_tiles[qc] = u_pool.tile([VPAD, QCHUNK], F32, tag="u", name=f"u{qc}")
                    u_ps = u_tiles[qc]
                    expS = exp_tiles[i]
                    nc.tensor.matmul(
                        u_ps,
                        lhsT=Fb[:, 2 * kp, :],
                        rhs=expS[:, 0:QCHUNK],
                        start=(kp == 0),
                        stop=False,
                    )
                    nc.tensor.matmul(
                        u_ps,
                        lhsT=Fb[:, 2 * kp + 1, :],
                        rhs=expS[:, QCHUNK : 2 * QCHUNK],
                        start=False,
                        stop=(kp == NBLK // 2 - 1),
                    )
                    exp_tiles[i] = None
                    if kp == NBLK // 2 - 1:
                        return (qc, u_ps)
                    return None

                def emit_epilogue(qc, u_ps):
                    # out2_natural[q, c'] = sum_c U'[c, q] Wv'[c, c'] comes
                    # out of the PE already transposed, denominator in col 64
                    u_sb = ep.tile([VPAD, QCHUNK], BF16, tag="usb")
                    nc.vector.tensor_copy(u_sb, u_ps)
                    o2 = o2_pool.tile([P, NQC, VPAD], F32, tag="o2")
                    for j in range(NQC):
                        nc.tensor.matmul(
                            o2[:, j, :],
                            lhsT=u_sb[:, j * P : (j + 1) * P],
                            rhs=Wv_b,
                            start=True,
                            stop=True,
                        )
                    rcp4 = ep.tile([P, NQC], F32, tag="rcp4")
                    nc.vector.reciprocal(rcp4, o2[:, :, C])
                    o_sb = ep.tile([P, NQC, C], F32, tag="osb")
                    # normalize+residual split across ACT and DVE and
                    # DMA'd per block pair to shorten the serial tail
                    for j in range(NQC):
                        if j % 2 == 0:
                            nc.scalar.mul(
                                o_sb[:, j, :], o2[:, j, 0:C],
                                rcp4[:, j : j + 1],
                            )
                        else:
                            nc.vector.tensor_scalar(
                                out=o_sb[:, j, :],
                                in0=o2[:, j, 0:C],
                                scalar1=rcp4[:, j : j + 1],
                                scalar2=None,
                                op0=mybir.AluOpType.mult,
                            )
                    for h in range(2):
                        blks = slice(2 * h, 2 * h + 2)
                        nc.vector.tensor_add(
                            o_sb[:, blks, :],
                            o_sb[:, blks, :],
                            F_sb[:, 4 * qc + 2 * h : 4 * qc + 2 * h + 2, :],
                        )
                        nc.sync.dma_start(
                            out=out_view[
                                :, 4 * qc + 2 * h : 4 * qc + 2 * h + 2, :
                            ],
                            in_=o_sb[:, blks, :],
                        )

                pending_ep = None
                for i in range(NT_TILES + DEPTH):
                    if i < NT_TILES:
                        emit_scores(i)
                    if pending_ep is not None:
                        emit_epilogue(*pending_ep)
                        pending_ep = None
                    if i >= DEPTH:
                        pending_ep = emit_u(i - DEPTH)
                if pending_ep is not None:
                    emit_epilogue(*pending_ep)

    nc.finalize()
    return nc


_NC_CACHE = None


def _get_nc() -> bass.Bass:
    global _NC_CACHE
    if _NC_CACHE is None:
        _NC_CACHE = build_nc()
    return _NC_CACHE


def run_spmd(F, W_M, W_N, W_V, **kwargs):
    """Run the SPMD kernel; returns the BassKernelResults (for profiling)."""
    nc = _get_nc()
    in_maps = [
        {
            "F": np.ascontiguousarray(F[i], dtype=np.float32),
            "W_M": np.ascontiguousarray(W_M, dtype=np.float32),
            "W_N": np.ascontiguousarray(W_N, dtype=np.float32),
            "W_V": np.ascontiguousarray(W_V, dtype=np.float32),
        }
        for i in range(B)
    ]
    return run_bass_kernel_spmd(nc, in_maps, core_ids=list(range(B)), **kwargs)


def kernel(F, W_M, W_N, W_V):
    res = run_spmd(F, W_M, W_N, W_V)
    return np.stack([r["out"] for r in res.results]).astype(np.float32)
